# revision 3
# baseline (speedup 1.0000x reference)
"""Trainium2 Bass kernel for nn_AttentionBlock (causal attn, softmax over the
QUERY axis (dim=1), post-softmax 1/sqrt(K) scale, residual add).

Sharding: data-parallel over batch B=8, one batch element per NeuronCore.

v3 design:
- all matmul operands bf16; v-projection and probs@V read in fp8e4m3 with
  DoubleRow (two contraction tiles per matmul).
- softmax max pass replaced by a per-key shift: shift_j = max(M_j, C) where
  M_j is the max over the strip's DIAGONAL chunk (captures the correlated
  q_j.k_j spike; C=52 floors typical columns).  One DVE reduce per strip.
- causal mask seeded into PSUM by a PE matmul (identity @ madd), then the
  diagonal logits accumulate onto it with start=False.
- logits PSUM banks live in one [128, 6*512] tensor; each strip's chunk
  pairs land in adjacent banks so ACT exps run 1024-wide (half the
  instruction overhead).  exp accum_out collects denominator parts; an
  epsilon slot guards degenerate columns; the probs conversion clamps
  min(E, 3.3e38) * rec so overflowed columns zero out instead of NaN.
- probs conversion p8 = E * rec on Pool (SBUF->SBUF); v evacuations on DVE;
  kq evacuations on ACT; output evacuation on DVE; fp32 store.
- residual add + 1/sqrt(K) on host.

Raw Block with manual semaphores; cross-engine deps are standalone wait_ge
with static thresholds; same-engine RAW pairs get explicit fences.  Loads
use one semaphore per tensor (DMA completions reorder).  Stores use two
parity semaphores so outst slot reuse pins the exact store.
"""

import math
import os
import sys
from contextlib import ExitStack

import numpy as np
import ml_dtypes

for _p in ("/opt/trn_rl_repo", "/root/.axon_site/_ro/trn_rl_repo"):
    if os.path.isdir(_p) and _p not in sys.path:
        sys.path.append(_p)

import concourse.bass as bass
from concourse import mybir
from concourse.bass_utils import run_bass_kernel_spmd

B = 8
D = 512
KS = 512
ND = 4
NT = 16
NCH = 4
T = 2048

F32 = mybir.dt.float32
BF16 = mybir.dt.bfloat16
FP8 = mybir.dt.float8e4
AOP = mybir.AluOpType
AFT = mybir.ActivationFunctionType
DR = mybir.MatmulPerfMode.DoubleRow

INV_SQRT_K = 1.0 / math.sqrt(KS)
CSHIFT = 52.0
EPS = 1e-36
CLAMP = 3.3e38

TRACE = False
LAST_RESULTS = None

FP8_V = True
FP8_L = False     # logits matmul in fp8 DoubleRow (kT/qT stored e4m3)
F8S = 16
# strips whose probs conversion runs on DVE instead of Pool
CONV_DVE = ()


def _c0(jt):
    return jt // 4


def build_nc(T=2048, fp8_v=None, f8s=None, conv_dve=None, has_bias=False,
             la_fn=None, fp8_l=None):
    if fp8_v is None:
        fp8_v = FP8_V
    if fp8_l is None:
        fp8_l = FP8_L
    if f8s is None:
        f8s = F8S
    if conv_dve is None:
        conv_dve = CONV_DVE
    conv_dve = set(conv_dve)
    assert T == 2048

    nc = bass.Bass("TRN2", target_bir_lowering=False, debug=False, num_devices=B)

    # ---- DRAM ----
    xb_d = nc.dram_tensor("xb", [128, ND, T], BF16, kind="ExternalInput")
    wk_d = nc.dram_tensor("wkb", [128, ND, KS], BF16, kind="ExternalInput")
    wq_d = nc.dram_tensor("wqb", [128, ND, KS], BF16, kind="ExternalInput")
    ma_d = nc.dram_tensor("madd", [128, 512], BF16, kind="ExternalInput")
    id_d = nc.dram_tensor("ident", [128, 128], BF16, kind="ExternalInput")
    if fp8_v:
        x8_d = nc.dram_tensor("x8", [128, ND, T], FP8, kind="ExternalInput")
        wv_d = nc.dram_tensor("wv8", [128, ND, KS], FP8, kind="ExternalInput")
    else:
        wv_d = nc.dram_tensor("wvb", [128, ND, KS], BF16, kind="ExternalInput")
    if has_bias:
        bk_d = nc.dram_tensor("bkc", [128, ND], F32, kind="ExternalInput")
        bq_d = nc.dram_tensor("bqc", [128, ND], F32, kind="ExternalInput")
        bv_d = nc.dram_tensor("bvb", [128, KS], F32, kind="ExternalInput")
    out_d = nc.dram_tensor("out", [T, KS], F32, kind="ExternalOutput")

    # ---- SBUF ----
    xb = nc.alloc_sbuf_tensor("xb_sb", [128, ND, T], BF16)
    wk = nc.alloc_sbuf_tensor("wk_sb", [128, ND, KS], BF16)
    wq = nc.alloc_sbuf_tensor("wq_sb", [128, ND, KS], BF16)
    if fp8_v:
        x8 = nc.alloc_sbuf_tensor("x8_sb", [128, ND, T], FP8)
        wv = nc.alloc_sbuf_tensor("wv_sb", [128, ND, KS], FP8)
    else:
        x8 = None
        wv = nc.alloc_sbuf_tensor("wv_sb", [128, ND, KS], BF16)
    kqdt = FP8 if fp8_l else BF16
    kT = nc.alloc_sbuf_tensor("kT", [128, ND, T], kqdt)
    qT = nc.alloc_sbuf_tensor("qT", [128, ND, T], kqdt)
    e_sbs = [
        nc.alloc_sbuf_tensor(f"e{jt}", [128, T - 128 * jt], BF16) for jt in range(NT)
    ]
    madd = nc.alloc_sbuf_tensor("madd_sb", [128, 512], BF16)
    ident = nc.alloc_sbuf_tensor("ident_sb", [128, 128], BF16)
    np8 = (f8s + 1) // 2
    p8_sbs = [
        nc.alloc_sbuf_tensor(f"p8_{jp}", [128, 2, T - 256 * jp], FP8)
        for jp in range(np8)
    ]
    v8p = None
    v_sb = None
    vp_sb = None
    if f8s > 0:
        v8p = nc.alloc_sbuf_tensor("v8p", [128, NT // 2, 2, KS], FP8)
    if f8s < NT:
        v_sb = nc.alloc_sbuf_tensor("v_sb", [128, NT, KS], BF16)
        vp_sb = nc.alloc_sbuf_tensor("vp_sb", [128, NT, KS], BF16)
    outst = nc.alloc_sbuf_tensor("outst", [128, 2, KS], F32)
    dparts = nc.alloc_sbuf_tensor("dparts", [128, 4, 3], F32)
    denom = nc.alloc_sbuf_tensor("denom", [128, 4], F32)
    rec = nc.alloc_sbuf_tensor("rec", [128, 4], F32)
    negm = nc.alloc_sbuf_tensor("negm", [128, 4], F32)
    negsh = nc.alloc_sbuf_tensor("negsh", [128, 4], F32)
    if has_bias:
        bkc = nc.alloc_sbuf_tensor("bkc_sb", [128, ND], F32)
        bqc = nc.alloc_sbuf_tensor("bqc_sb", [128, ND], F32)
        bvb = nc.alloc_sbuf_tensor("bvb_sb", [128, KS], F32)

    # PSUM: logits banks 0..5 contiguous (super-bank exps); read banks 6,7
    psL = nc.alloc_psum_tensor("psL", [128, 6 * 512], F32)
    psR = [nc.alloc_psum_tensor(f"psR{i}", [128, 512], F32) for i in range(2)]

    def LB(b, lo=0, hi=512):  # logits bank slice
        return psL[:, 512 * b + lo : 512 * b + hi]

    # ================= static plan =================
    load_names = []
    if fp8_v:
        load_names += ["wv", "x8_0", "ident", "wk", "xb0", "wq", "xb1", "x8_1",
                       "xb2", "x8_2", "xb3", "x8_3", "madd"]
    else:
        load_names += ["wv", "xb0", "ident", "wk", "wq", "xb1", "xb2", "xb3",
                       "madd"]
    if has_bias:
        load_names += ["bk", "bq", "bv"]

    # ---- PE groups ----
    VG, KG, QG, KQSEQ = {}, {}, {}, {}
    pe = 0
    kqseq = 0
    p1_order = []
    for ic in range(NCH):
        for jt in range(4 * ic, 4 * ic + 4):
            pe += 1
            VG[jt] = pe
            p1_order.append(("v", jt))
        for kind in ("k", "q"):
            for kt in range(ND):
                pe += 1
                kqseq += 1
                (KG if kind == "k" else QG)[(kt, ic)] = pe
                KQSEQ[(kind, kt, ic)] = kqseq
                p1_order.append((kind, kt, ic))

    # v bank rotation: stages 0-2 use [L0, L1, R0, R1][jt%4]; stage 3 R-only
    def vbank(jt):
        if jt >= 12:
            return ("R", jt % 2)
        return [("L", 0), ("L", 1), ("R", 0), ("R", 1)][jt % 4]

    def vbank_prev(jt):
        if jt >= 12:
            return jt - 2
        return jt - 4 if jt >= 4 else None

    # fused order
    if la_fn is None:
        la_fn = lambda j: 2
    fused_order = []
    emitted = set()
    for m in range(NT):
        fused_order.append(("L", m))
        for j in range(NT):
            if j not in emitted and j + la_fn(j) <= m:
                fused_order.append(("R", j))
                emitted.add(j)
    for j in range(NT):
        if j not in emitted:
            fused_order.append(("R", j))
            emitted.add(j)

    LG, RG = {}, {}
    chunk_list = []
    CH_G = {}
    g = 0
    for kind, m in fused_order:
        if kind == "L":
            for ic in range(_c0(m), NCH):
                pe += 1
                LG[(m, ic)] = pe
                CH_G[(m, ic)] = g
                chunk_list.append((m, ic))
                g += 1
        else:
            pe += 1
            RG[m] = pe

    # super-exp grouping: runs of <=2 chunks in adjacent banks
    SUPERS = {}
    for m in range(NT):
        c0 = _c0(m)
        ics = list(range(c0, NCH))
        runs = []
        i = 0
        while i < len(ics):
            pair = ics[i : i + 2]
            if len(pair) == 2:
                b0 = CH_G[(m, pair[0])] % 6
                b1 = CH_G[(m, pair[1])] % 6
                if b1 != b0 + 1:
                    pair = pair[:1]
            runs.append(pair)
            i += len(pair)
        SUPERS[m] = runs

    # ---- ACT plan ----
    AKE, AQE = {}, {}
    ac = 0
    for ic in range(NCH):
        for kind in ("k", "q"):
            for kt in range(ND):
                ac += 1
                (AKE if kind == "k" else AQE)[(kt, ic)] = ac
    EXPI = {}
    EXP_END = {}
    for m in range(NT):
        for run in SUPERS[m]:
            ac += 1
            for ic in run:
                EXPI[(m, ic)] = ac
        EXP_END[m] = ac
    N_AC = ac

    def kqevac_idx(seq):
        for (kind, kt, ic), s_ in KQSEQ.items():
            if s_ == seq:
                return (AKE if kind == "k" else AQE)[(kt, ic)]
        raise KeyError(seq)

    # ---- DVE plan ----
    DVE_VE, OCI, RSUMI, RECI, VPI, NSH = {}, {}, {}, {}, {}, {}
    dve_prog = []
    dv = 0

    def demit(tag, *args):
        nonlocal dv
        dv += 1
        dve_prog.append((tag, dv) + args)
        return dv

    demit("memset_eps")
    for jt in range(NT):
        DVE_VE[jt] = demit("vevac", jt)
    # nmax/nshift LEAD by one strip so per-strip softmax chains pipeline
    NLEAD = 1
    for m in range(min(NLEAD, NT)):
        demit("nmax", m)
        NSH[m] = demit("nshift", m)
    for m in range(NT):
        if m + NLEAD < NT:
            demit("nmax", m + NLEAD)
            NSH[m + NLEAD] = demit("nshift", m + NLEAD)
        if m - 2 >= 0:
            OCI[m - 2] = demit("outcopy", m - 2)
        RSUMI[m] = demit("rsum", m)
        RECI[m] = demit("recip", m)
        if m < f8s and m in conv_dve:
            VPI[m] = demit("conv", m)
        elif m >= f8s:
            VPI[m] = demit("vp", m)
        else:
            VPI[m] = RECI[m]
    for j in (NT - 2, NT - 1):
        OCI[j] = demit("outcopy", j)
    N_DV = dv

    # ---- Pool plan ----
    PCONV = {}
    pl_prog = []
    plc = 0
    if f8s > 0:
        for jp in range(np8):
            plc += 1
            pl_prog.append(("hole", plc, jp))
        for m in range(f8s):
            if m not in conv_dve:
                plc += 1
                PCONV[m] = plc
                pl_prog.append(("conv", plc, m))
    N_PL = plc

    def bank_free(gg):
        if gg >= 6:
            pj, pic = chunk_list[gg - 6]
            return ("AC", EXPI[(pj, pic)])
        if gg < 2:
            return ("DV", DVE_VE[8 + gg])   # L0/L1 last v users: v(8), v(9)
        return ("AC", kqevac_idx(27 + gg))  # banks 2..5: kq seq 29..32

    def r_gate(j):
        need_dv = VPI[j]
        need_pl = 0
        jj = min(j, f8s - 1)
        if f8s > 0 and jj not in conv_dve:
            need_pl = PCONV[jj]
        return need_dv, need_pl

    with ExitStack() as _stack:
        sLD = {n: _stack.enter_context(nc.semaphore(f"sL_{n}")) for n in load_names}
        sPE = _stack.enter_context(nc.semaphore("sPE"))
        sAC = _stack.enter_context(nc.semaphore("sAC"))
        sDV = _stack.enter_context(nc.semaphore("sDV"))
        sPL = _stack.enter_context(nc.semaphore("sPL"))
        sST0 = _stack.enter_context(nc.semaphore("sST0"))
        sST1 = _stack.enter_context(nc.semaphore("sST1"))
        sSTp = [sST0, sST1]
        block = _stack.enter_context(nc.Block())

        # ================= SP: DMA =================
        @block.sync
        def _(sp):
            def ldchunk(dst, src, ic, sem):
                sp.dma_start(
                    out=dst[:, :, 512 * ic : 512 * (ic + 1)],
                    in_=src.ap()[:, :, 512 * ic : 512 * (ic + 1)],
                ).then_inc(sem, 16)

            for name in load_names:
                sem = sLD[name]
                if name == "wv":
                    sp.dma_start(out=wv[:, :, :], in_=wv_d.ap()).then_inc(sem, 16)
                elif name == "wk":
                    sp.dma_start(out=wk[:, :, :], in_=wk_d.ap()).then_inc(sem, 16)
                elif name == "wq":
                    sp.dma_start(out=wq[:, :, :], in_=wq_d.ap()).then_inc(sem, 16)
                elif name == "madd":
                    sp.dma_start(out=madd[:, :], in_=ma_d.ap()).then_inc(sem, 16)
                elif name == "ident":
                    sp.dma_start(out=ident[:, :], in_=id_d.ap()).then_inc(sem, 16)
                elif name.startswith("xb"):
                    ldchunk(xb, xb_d, int(name[2]), sem)
                elif name.startswith("x8_"):
                    ldchunk(x8, x8_d, int(name[3]), sem)
                elif name == "bk":
                    sp.dma_start(out=bkc[:, :], in_=bk_d.ap()).then_inc(sem, 16)
                elif name == "bq":
                    sp.dma_start(out=bqc[:, :], in_=bq_d.ap()).then_inc(sem, 16)
                elif name == "bv":
                    sp.dma_start(out=bvb[:, :], in_=bv_d.ap()).then_inc(sem, 16)
                else:
                    raise KeyError(name)

            out_ap = out_d.ap()
            for j in range(NT):
                sp.wait_ge(sDV, OCI[j])
                sp.dma_start(
                    out=out_ap[128 * j : 128 * (j + 1), :],
                    in_=outst[:, j % 2, :],
                ).then_inc(sSTp[j % 2], 16)
            sp.wait_ge(sST0, 16 * (NT // 2))
            sp.wait_ge(sST1, 16 * (NT // 2))

        # ================= PE =================
        @block.tensor
        def _(te):
            waited = set()

            def ldwait(*names):
                for n in names:
                    if n not in waited:
                        te.wait_ge(sLD[n], 16)
                        waited.add(n)

            for item in p1_order:
                if item[0] == "v":
                    jt = item[1]
                    ic = jt // 4
                    ldwait("wv", ("x8_%d" % ic) if fp8_v else ("xb%d" % ic))
                    pv = vbank_prev(jt)
                    if pv is not None:
                        te.wait_ge(sDV, DVE_VE[pv])
                    kindb, idx = vbank(jt)
                    bank = psR[idx][:, :] if kindb == "R" else LB(idx)
                    if fp8_v:
                        for u in range(2):
                            mm = te.matmul(
                                bank,
                                lhsT=x8[:, 2 * u : 2 * u + 2, 128 * jt : 128 * (jt + 1)],
                                rhs=wv[:, 2 * u : 2 * u + 2, :],
                                start=(u == 0),
                                stop=(u == 1),
                                perf_mode=DR,
                            )
                            if u == 1:
                                mm.then_inc(sPE, 1)
                    else:
                        for dt_ in range(ND):
                            mm = te.matmul(
                                bank,
                                lhsT=xb[:, dt_, 128 * jt : 128 * (jt + 1)],
                                rhs=wv[:, dt_, :],
                                start=(dt_ == 0),
                                stop=(dt_ == ND - 1),
                            )
                            if dt_ == ND - 1:
                                mm.then_inc(sPE, 1)
                else:
                    kind, kt, ic = item
                    wsb = wk if kind == "k" else wq
                    ldwait("wk" if kind == "k" else "wq", "xb%d" % ic)
                    seq = KQSEQ[(kind, kt, ic)]
                    if seq > 4:
                        te.wait_ge(sAC, kqevac_idx(seq - 4))
                    bank = LB(2 + (seq - 1) % 4)
                    for dt_ in range(ND):
                        mm = te.matmul(
                            bank,
                            lhsT=wsb[:, dt_, 128 * kt : 128 * (kt + 1)],
                            rhs=xb[:, dt_, 512 * ic : 512 * (ic + 1)],
                            start=(dt_ == 0),
                            stop=(dt_ == ND - 1),
                        )
                        if dt_ == ND - 1:
                            mm.then_inc(sPE, 1)

            # ---- fused ----
            ldwait("madd", "ident")
            for kind, m in fused_order:
                if kind == "L":
                    c0 = _c0(m)
                    for ic in range(c0, NCH):
                        gg = CH_G[(m, ic)]
                        b = gg % 6
                        kindw, fidx = bank_free(gg)
                        need_ac = max(AKE[(ND - 1, c0)], AQE[(ND - 1, ic)])
                        if kindw == "AC":
                            need_ac = max(need_ac, fidx)
                            te.wait_ge(sAC, need_ac)
                        else:
                            te.wait_ge(sAC, need_ac)
                            te.wait_ge(sDV, fidx)
                        diag = ic == c0
                        if diag:
                            w0 = 512 * (c0 + 1) - 128 * m
                            te.matmul(
                                LB(b, 512 - w0, 512),
                                lhsT=ident[:, :],
                                rhs=madd[:, 0:w0],
                                start=True,
                                stop=False,
                                skip_group_check=True,
                            )
                            ilo = 128 * m
                            out = LB(b, 512 - w0, 512)
                        else:
                            ilo = 512 * ic
                            out = LB(b)
                        if fp8_l:
                            for u in range(2):
                                mm = te.matmul(
                                    out,
                                    lhsT=kT[:, 2 * u : 2 * u + 2, 128 * m : 128 * (m + 1)],
                                    rhs=qT[:, 2 * u : 2 * u + 2, ilo : 512 * (ic + 1)],
                                    start=(False if diag else u == 0),
                                    stop=(u == 1),
                                    perf_mode=DR,
                                    skip_group_check=True,
                                )
                                if u == 1:
                                    mm.then_inc(sPE, 1)
                        else:
                            for kt in range(ND):
                                mm = te.matmul(
                                    out,
                                    lhsT=kT[:, kt, 128 * m : 128 * (m + 1)],
                                    rhs=qT[:, kt, ilo : 512 * (ic + 1)],
                                    start=(False if diag else kt == 0),
                                    stop=(kt == ND - 1),
                                    skip_group_check=diag,
                                )
                                if kt == ND - 1:
                                    mm.then_inc(sPE, 1)
                else:
                    j = m
                    need_dv, need_pl = r_gate(j)
                    if j >= 2:
                        need_dv = max(need_dv, OCI[j - 2])
                    te.wait_ge(sDV, need_dv)
                    if need_pl:
                        te.wait_ge(sPL, need_pl)
                    if j < 2:
                        te.wait_ge(sDV, DVE_VE[14 + j])  # psR last v users
                    bank = psR[j % 2][:, :]
                    nf8 = min(j + 1, f8s)
                    npair = (nf8 + 1) // 2
                    total_mm = npair + max(0, j + 1 - f8s)
                    mi = 0
                    for jp in range(npair):
                        rel = 128 * j - 256 * jp
                        if 2 * jp + 1 < f8s:
                            mm = te.matmul(
                                bank,
                                lhsT=p8_sbs[jp][:, :, rel : rel + 128],
                                rhs=v8p[:, jp, :, :],
                                start=(mi == 0),
                                stop=(mi == total_mm - 1),
                                perf_mode=DR,
                                skip_group_check=True,
                            )
                        else:
                            mm = te.matmul(
                                bank,
                                lhsT=p8_sbs[jp][:, 0, rel : rel + 128],
                                rhs=v8p[:, jp, 0, :],
                                start=(mi == 0),
                                stop=(mi == total_mm - 1),
                                skip_group_check=True,
                            )
                        mi += 1
                        if mi == total_mm:
                            mm.then_inc(sPE, 1)
                    for s_ in range(f8s, j + 1):
                        mm = te.matmul(
                            bank,
                            lhsT=e_sbs[s_][:, 128 * (j - s_) : 128 * (j - s_ + 1)],
                            rhs=vp_sb[:, s_, :],
                            start=(mi == 0),
                            stop=(mi == total_mm - 1),
                            skip_group_check=True,
                        )
                        mi += 1
                        if mi == total_mm:
                            mm.then_inc(sPE, 1)

        # ================= ACT =================
        @block.scalar
        def _(ac_):
            nac = 0

            def inc(x):
                nonlocal nac
                nac += 1
                x.then_inc(sAC, 1)

            for ic in range(NCH):
                for kind in ("k", "q"):
                    dstT = kT if kind == "k" else qT
                    for kt in range(ND):
                        seq = KQSEQ[(kind, kt, ic)]
                        ac_.wait_ge(sPE, (KG if kind == "k" else QG)[(kt, ic)])
                        bank = LB(2 + (seq - 1) % 4)
                        if has_bias:
                            bias = bkc if kind == "k" else bqc
                            a = ac_.activation(
                                out=dstT[:, kt, 512 * ic : 512 * (ic + 1)],
                                in_=bank,
                                func=AFT.Identity,
                                bias=bias[:, kt : kt + 1],
                                scale=1.0,
                            )
                        else:
                            a = ac_.copy(
                                out=dstT[:, kt, 512 * ic : 512 * (ic + 1)],
                                in_=bank,
                            )
                        inc(a)
                        assert nac == (AKE if kind == "k" else AQE)[(kt, ic)]

            for m in range(NT):
                c0 = _c0(m)
                w0 = 512 * (c0 + 1) - 128 * m
                n_sup = len(SUPERS[m])
                lo_slot = 2 - n_sup
                for si, run in enumerate(SUPERS[m]):
                    last_ic = run[-1]
                    ac_.wait_ge(sPE, LG[(m, last_ic)])
                    if si == 0:
                        ac_.wait_ge(sDV, NSH[m])
                        if m >= 4:
                            ac_.wait_ge(sDV, RSUMI[m - 4])
                    b0 = CH_G[(m, run[0])] % 6
                    if run[0] == c0:
                        lo = 512 - w0
                        ecol = 0
                        width = w0 + 512 * (len(run) - 1)
                    else:
                        lo = 0
                        ecol = 512 * run[0] - 128 * m
                        width = 512 * len(run)
                    inc(
                        ac_.activation(
                            out=e_sbs[m][:, ecol : ecol + width],
                            in_=psL[:, 512 * b0 + lo : 512 * b0 + lo + width],
                            func=AFT.Exp,
                            bias=negsh[:, m % 4 : m % 4 + 1],
                            scale=1.0,
                            accum_out=dparts[:, m % 4, lo_slot + si : lo_slot + si + 1],
                        )
                    )
                    assert nac == EXPI[(m, run[0])]
            assert nac == N_AC

        # ================= DVE =================
        @block.vector
        def _(ve):
            ndv = 0

            def inc(x):
                nonlocal ndv
                ndv += 1
                x.then_inc(sDV, 1)

            for op in dve_prog:
                tag, idx = op[0], op[1]
                if tag == "memset_eps":
                    inc(ve.memset(dparts[:, :, 2:3], EPS))
                elif tag == "vevac":
                    jt = op[2]
                    ve.wait_ge(sPE, VG[jt])
                    kindb, bidx = vbank(jt)
                    bank = psR[bidx][:, :] if kindb == "R" else LB(bidx)
                    if jt < f8s:
                        dst = v8p[:, jt // 2, jt % 2, :]
                    else:
                        dst = v_sb[:, jt, :]
                    inc(
                        ve.tensor_scalar(
                            out=dst, in0=bank, scalar1=0.0, scalar2=None,
                            op0=AOP.add,
                        )
                    )
                elif tag == "nmax":
                    m = op[2]
                    gg = CH_G[(m, _c0(m))]
                    w0 = 512 * (_c0(m) + 1) - 128 * m
                    ve.wait_ge(sPE, LG[(m, _c0(m))])
                    inc(
                        ve.reduce_max(
                            negm[:, m % 4 : m % 4 + 1],
                            LB(gg % 6, 512 - w0, 512),
                            axis=mybir.AxisListType.X,
                            negate=True,
                        )
                    )
                elif tag == "nshift":
                    m = op[2]
                    ve.wait_ge(sDV, NSH[m] - 1)  # same-engine RAW fence
                    if m >= 4:
                        ve.wait_ge(sAC, EXP_END[m - 4])  # negsh slot reuse
                    inc(
                        ve.tensor_scalar(
                            out=negsh[:, m % 4 : m % 4 + 1],
                            in0=negm[:, m % 4 : m % 4 + 1],
                            scalar1=-CSHIFT,
                            scalar2=None,
                            op0=AOP.min,
                        )
                    )
                elif tag == "outcopy":
                    j = op[2]
                    ve.wait_ge(sPE, RG[j])
                    if j >= 2:
                        ve.wait_ge(sSTp[j % 2], 16 * (j // 2))
                    inc(
                        ve.tensor_scalar(
                            out=outst[:, j % 2, :],
                            in0=psR[j % 2][:, :],
                            scalar1=0.0,
                            scalar2=None,
                            op0=AOP.add,
                        )
                    )
                elif tag == "rsum":
                    m = op[2]
                    ve.wait_ge(sAC, EXP_END[m])
                    ve.wait_ge(sDV, 1)  # eps memset
                    lo_slot = 2 - len(SUPERS[m])
                    inc(
                        ve.tensor_reduce(
                            out=denom[:, m % 4 : m % 4 + 1],
                            in_=dparts[:, m % 4, lo_slot : 3],
                            axis=mybir.AxisListType.X,
                            op=AOP.add,
                        )
                    )
                elif tag == "recip":
                    m = op[2]
                    ve.wait_ge(sDV, RSUMI[m])  # same-engine RAW fence
                    inc(
                        ve.reciprocal(
                            rec[:, m % 4 : m % 4 + 1], denom[:, m % 4 : m % 4 + 1]
                        )
                    )
                elif tag == "conv":
                    m = op[2]
                    ve.wait_ge(sDV, RECI[m])  # same-engine RAW fence
                    jp, tt = m // 2, m % 2
                    cols = T - 128 * m
                    inc(
                        ve.tensor_scalar(
                            out=p8_sbs[jp][:, tt, 128 * tt : 128 * tt + cols],
                            in0=e_sbs[m][:, :],
                            scalar1=CLAMP,
                            scalar2=rec[:, m % 4 : m % 4 + 1],
                            op0=AOP.min,
                            op1=AOP.mult,
                        )
                    )
                elif tag == "vp":
                    m = op[2]
                    ve.wait_ge(sDV, RECI[m])  # same-engine RAW fence
                    if has_bias:
                        ve.tensor_tensor(
                            out=vp_sb[:, m, :], in0=v_sb[:, m, :],
                            in1=bvb[:, :], op=AOP.add,
                        )
                        inc(
                            ve.tensor_scalar(
                                out=vp_sb[:, m, :], in0=vp_sb[:, m, :],
                                scalar1=CLAMP, scalar2=rec[:, m % 4 : m % 4 + 1],
                                op0=AOP.min, op1=AOP.mult,
                            )
                        )
                    else:
                        inc(
                            ve.tensor_scalar(
                                out=vp_sb[:, m, :], in0=v_sb[:, m, :],
                                scalar1=CLAMP, scalar2=rec[:, m % 4 : m % 4 + 1],
                                op0=AOP.min, op1=AOP.mult,
                            )
                        )
                else:
                    raise KeyError(tag)
            assert ndv == N_DV

        # ================= Pool =================
        if N_PL:
            @block.gpsimd
            def _(pl):
                npl = 0

                def inc(x):
                    nonlocal npl
                    npl += 1
                    x.then_inc(sPL, 1)

                for op in pl_prog:
                    tag, idx = op[0], op[1]
                    if tag == "hole":
                        jp = op[2]
                        inc(pl.memset(p8_sbs[jp][:, 1, 0:128], 0.0))
                    elif tag == "conv":
                        m = op[2]
                        jp, tt = m // 2, m % 2
                        cols = T - 128 * m
                        pl.wait_ge(sDV, RECI[m])
                        inc(
                            pl.tensor_scalar(
                                out=p8_sbs[jp][:, tt, 128 * tt : 128 * tt + cols],
                                in0=e_sbs[m][:, :],
                                scalar1=CLAMP,
                                scalar2=rec[:, m % 4 : m % 4 + 1],
                                op0=AOP.min,
                                op1=AOP.mult,
                            )
                        )
                    else:
                        raise KeyError(tag)
                assert npl == N_PL

    nc.finalize()
    return nc


def _prep_w(W):
    return np.ascontiguousarray(W.T.reshape(ND, 128, KS).transpose(1, 0, 2))


def kernel(x, Wk, bk, Wq, bq, Wv, bv):
    global LAST_RESULTS
    x = np.asarray(x, dtype=np.float32)
    Wk = np.asarray(Wk, dtype=np.float32)
    Wq = np.asarray(Wq, dtype=np.float32)
    Wv = np.asarray(Wv, dtype=np.float32)
    bk = np.asarray(bk, dtype=np.float32)
    bq = np.asarray(bq, dtype=np.float32)
    bv = np.asarray(bv, dtype=np.float32)
    has_bias = bool(np.any(bk) or np.any(bq) or np.any(bv))

    wkb = _prep_w(Wk).astype(ml_dtypes.bfloat16)
    wqb = _prep_w(Wq).astype(ml_dtypes.bfloat16)
    wvf = _prep_w(Wv)
    fp8_v = FP8_V and not has_bias
    f8s = 0 if has_bias else F8S
    if fp8_v:
        wv8 = wvf.astype(ml_dtypes.float8_e4m3fn)
        wvb = None
    else:
        wv8 = None
        wvb = wvf.astype(ml_dtypes.bfloat16)

    xrows = [
        np.ascontiguousarray(x[b].T.reshape(ND, 128, T).transpose(1, 0, 2))
        for b in range(B)
    ]
    xbs = [xr.astype(ml_dtypes.bfloat16) for xr in xrows]
    x8s = [xr.astype(ml_dtypes.float8_e4m3fn) for xr in xrows] if fp8_v else [None] * B

    pcol = np.arange(128, dtype=np.float32)[:, None]
    xx = np.arange(512, dtype=np.float32)[None, :]
    madd = np.ascontiguousarray(
        np.where(xx >= pcol, 0.0, -3.0e38).astype(ml_dtypes.bfloat16)
    )
    ident = np.ascontiguousarray(
        np.eye(128, dtype=np.float32).astype(ml_dtypes.bfloat16)
    )

    def host_inputs(b):
        d_ = dict(xb=xbs[b], wkb=wkb, wqb=wqb, madd=madd, ident=ident)
        if fp8_v:
            d_["x8"] = x8s[b]
            d_["wv8"] = wv8
        else:
            d_["wvb"] = wvb
        if has_bias:
            d_["bkc"] = np.ascontiguousarray(bk.reshape(ND, 128).T.astype(np.float32))
            d_["bqc"] = np.ascontiguousarray(bq.reshape(ND, 128).T.astype(np.float32))
            d_["bvb"] = np.ascontiguousarray(
                np.broadcast_to(bv.astype(np.float32), (128, KS)).copy()
            )
        return d_

    nc = build_nc(T, fp8_v=fp8_v, f8s=f8s, has_bias=has_bias)
    in_maps = [host_inputs(b) for b in range(B)]
    res = None
    last_exc = None
    for attempt in range(3):
        try:
            res = run_bass_kernel_spmd(nc, in_maps, list(range(B)), trace=TRACE)
            break
        except Exception as e:
            last_exc = e
            import time as _time
            _time.sleep(10)
            nc = build_nc(T, fp8_v=fp8_v, f8s=f8s, has_bias=has_bias)
    if res is None:
        raise last_exc
    LAST_RESULTS = res
    read = np.stack([np.asarray(res.results[b]["out"]) for b in range(B)], axis=0)
    return (x + read * np.float32(INV_SQRT_K)).astype(np.float32)
